# revision 1
# baseline (speedup 1.0000x reference)
"""GCN 2-layer (GCNConv + LayerNorm + ReLU + residual) on 8 Trainium2 NeuronCores.

Strategy (dst-sharded message passing):
  - Host packs nodes into (core, block, slot): 8 cores x 98 blocks x 128 slots,
    snake-balanced by in-degree so every block has ~equal edge count.
  - Device, per core:
      phase 0: in-degree via ELL row-reduce -> dis = rsqrt(deg+1)
      phase A1: full layer-1 gather table t1 = dis * (x @ W1) in bf16
                (computed redundantly by every core; input x is replicated)
      phase B1: per own block: gather 128-edge chunks from t1 (indirect DMA),
                build selector S[e,d] = (iota==dslot)*|ew| on DVE, accumulate
                S^T @ gathered in PSUM; epilogue: +self-loop, *dis, +bias,
                LayerNorm, ReLU, +residual -> h1; then transpose + matmul
                -> xws2 shard rows (layer-2 table shard)
      AllGather xws2 shards -> t2
      phase B2: same aggregation/epilogue from t2 -> output shard
  - Host reassembles/un-permutes the 8 output shards.
"""
import os
import sys

import numpy as np

sys.path.insert(0, "/opt/trn_rl_repo")
import ml_dtypes  # noqa: E402

N = 100000
E = 3200000
D = 256
NC = 8
BPC = 98                 # blocks per core
NPC = BPC * 128          # node slots per core (12544)
NROWS = NC * NPC         # table rows (100352)
K = 34                   # chunks per block (capacity K*128 edges)
C = BPC * K              # chunks per core
KDEG = 80                # ELL width for degree computation
NBLK = NC * BPC          # 784 total blocks
LN_EPS = 1e-5
PAD_SLOT = 255.0

_cache = {}
LAST_RESULT = None


# ----------------------------------------------------------------------------
# host-side packing (pure indexing / layout, no tensor math)
# ----------------------------------------------------------------------------
def _pack(edge_index):
    src = np.asarray(edge_index[0], dtype=np.int64)
    dst = np.asarray(edge_index[1], dtype=np.int64)
    indeg = np.bincount(dst, minlength=N)

    order = np.argsort(-indeg, kind="stable")
    rank = np.empty(N, dtype=np.int64)
    rank[order] = np.arange(N)

    q, t = np.divmod(rank, NC)
    core_of = np.where(q % 2 == 0, t, NC - 1 - t)
    lr = q  # local rank within core, 0..12499
    qq, tt = np.divmod(lr, BPC)
    block_of = np.where(qq % 2 == 0, tt, BPC - 1 - tt)
    slot_of = qq
    assert slot_of.max() < 128

    gblock_of = core_of * BPC + block_of          # global block id
    pos2_of = core_of * NPC + block_of * 128 + slot_of  # global t2 layout

    # per-edge dst attributes
    e_core = core_of[dst]
    e_block = block_of[dst]
    e_dslot = slot_of[dst]

    # within-node rank for ELL
    by_dst = np.argsort(dst, kind="stable")
    dst_sorted = dst[by_dst]
    starts = np.searchsorted(dst_sorted, np.arange(N))
    rnk_sorted = np.arange(E) - starts[dst_sorted]
    rnk = np.empty(E, dtype=np.int64)
    rnk[by_dst] = rnk_sorted
    assert rnk.max() < KDEG, f"max in-degree {rnk.max()+1} exceeds KDEG={KDEG}"

    per_core = []
    for c in range(NC):
        # core-local block order: own blocks first, then others ascending
        oth = core_of - (core_of > c).astype(np.int64)
        lb_of = np.where(core_of == c, block_of, BPC + oth * BPC + block_of)
        pos1_of = lb_of * 128 + slot_of

        mask = e_core == c
        eb = e_block[mask]
        es = e_dslot[mask]
        ew_c = None  # filled later (values)
        esrc = src[mask]

        idx = np.argsort(eb, kind="stable")
        eb_s = eb[idx]
        es_s = es[idx]
        esrc_s = esrc[idx]
        bc = np.bincount(eb_s, minlength=BPC)
        assert bc.max() <= K * 128, f"block overflow {bc.max()} > {K*128}"
        bounds = np.concatenate([[0], np.cumsum(bc)])
        wpos = np.arange(len(eb_s)) - bounds[eb_s]
        chunk = eb_s * K + wpos // 128
        lane = wpos % 128

        per_core.append(
            dict(
                lb_of=lb_of, pos1_of=pos1_of, mask=mask, idx=idx,
                eb_s=eb_s, es_s=es_s, esrc_s=esrc_s, chunk=chunk, lane=lane,
            )
        )

    return dict(
        core_of=core_of, block_of=block_of, slot_of=slot_of,
        gblock_of=gblock_of, pos2_of=pos2_of, rnk=rnk, dst=dst, src=src,
        per_core=per_core,
    )


def _host_inputs(pk, x, edge_weight):
    x = np.asarray(x, dtype=np.float32)
    ew = np.asarray(edge_weight, dtype=np.float32).reshape(-1)
    src, dst, rnk = pk["src"], pk["dst"], pk["rnk"]
    core_of, slot_of = pk["core_of"], pk["slot_of"]

    ins = []
    for c in range(NC):
        pc = pk["per_core"][c]
        lb_of, pos1_of, mask, idx = pc["lb_of"], pc["pos1_of"], pc["mask"], pc["idx"]
        chunk, lane = pc["chunk"], pc["lane"]
        es_s, esrc_s = pc["es_s"], pc["esrc_s"]

        # x permuted to this core's t1 layout, then transposed per block
        x_perm = np.zeros((NROWS, D), np.float32)
        x_perm[pos1_of] = x
        xTb = np.ascontiguousarray(
            x_perm.reshape(NBLK, 128, D).transpose(0, 2, 1)
        )  # [784, 256, 128]
        x_own = np.ascontiguousarray(x_perm[:NPC])  # [12544, 256]

        # edge metadata in [128, C] lane-major layout
        ew_s = ew[mask][idx]
        dslot_a = np.full((128, C), PAD_SLOT, np.float32)
        ewraw_a = np.zeros((128, C), np.float32)
        srcp1_a = np.zeros((128, C), np.int32)
        srcp2_a = np.zeros((128, C), np.int32)
        dslot_a[lane, chunk] = es_s
        ewraw_a[lane, chunk] = ew_s
        srcp1_a[lane, chunk] = pos1_of[esrc_s]
        srcp2_a[lane, chunk] = pk["pos2_of"][esrc_s]

        # ELL for degrees, columns in this core's block order (own first)
        ell = np.zeros((128, NBLK * KDEG), np.float32)
        ell[slot_of[dst], lb_of[dst] * KDEG + rnk] = ew

        ins.append(
            dict(
                xTb=xTb, x_own=x_own,
                dslot=dslot_a, ewraw=ewraw_a, srcp1=srcp1_a, srcp2=srcp2_a,
                ell=ell,
            )
        )
    return ins


# ----------------------------------------------------------------------------
# device program
# ----------------------------------------------------------------------------
def _build_program():
    import concourse.bacc as bacc
    import concourse.tile as tile
    from concourse import mybir
    from concourse.bass import IndirectOffsetOnAxis

    BF = mybir.dt.bfloat16
    F32 = mybir.dt.float32
    I32 = mybir.dt.int32
    AO = mybir.AluOpType
    AF = mybir.ActivationFunctionType
    AX = __import__("bass_rust").AxisListType

    nc = bacc.Bacc("TRN2", target_bir_lowering=False, debug=False, num_devices=NC)

    # inputs
    xTb = nc.dram_tensor("xTb", [NBLK, D, 128], F32, kind="ExternalInput")
    x_own = nc.dram_tensor("x_own", [NPC, D], F32, kind="ExternalInput")
    dslot = nc.dram_tensor("dslot", [128, C], F32, kind="ExternalInput")
    ewraw = nc.dram_tensor("ewraw", [128, C], F32, kind="ExternalInput")
    srcp1 = nc.dram_tensor("srcp1", [128, C], I32, kind="ExternalInput")
    srcp2 = nc.dram_tensor("srcp2", [128, C], I32, kind="ExternalInput")
    ell = nc.dram_tensor("ell", [128, NBLK * KDEG], F32, kind="ExternalInput")
    iota_in = nc.dram_tensor("iota", [128, 128], BF, kind="ExternalInput")
    W1_in = nc.dram_tensor("W1", [D, D], F32, kind="ExternalInput")
    W2_in = nc.dram_tensor("W2", [D, D], F32, kind="ExternalInput")
    b1bc = nc.dram_tensor("b1bc", [128, D], F32, kind="ExternalInput")
    g1bc = nc.dram_tensor("g1bc", [128, D], F32, kind="ExternalInput")
    e1bc = nc.dram_tensor("e1bc", [128, D], F32, kind="ExternalInput")
    b2bc = nc.dram_tensor("b2bc", [128, D], F32, kind="ExternalInput")
    g2bc = nc.dram_tensor("g2bc", [128, D], F32, kind="ExternalInput")
    e2bc = nc.dram_tensor("e2bc", [128, D], F32, kind="ExternalInput")

    h_out = nc.dram_tensor("h_out", [NPC, D], F32, kind="ExternalOutput")

    # internal DRAM (exposed as outputs for debugging; harmless)
    t1 = nc.dram_tensor("t1", [NROWS, D], BF, kind="ExternalOutput")
    h1 = nc.dram_tensor("h1", [NPC, D], F32, kind="ExternalOutput")
    xws2 = nc.dram_tensor("xws2", [NPC, D], BF)
    xws2d = nc.dram_tensor("xws2d", [NPC, D], BF, kind="ExternalOutput")
    t2 = nc.dram_tensor("t2", [NROWS, D], BF, addr_space="Shared")
    dis_dump = nc.dram_tensor("dis_dump", [128, NBLK], F32, kind="ExternalOutput")

    with tile.TileContext(nc) as tc:
        with (
            tc.tile_pool(name="meta", bufs=1) as meta,
            tc.tile_pool(name="ellp", bufs=3) as ellp,
            tc.tile_pool(name="a1", bufs=4) as a1p,
            tc.tile_pool(name="gat", bufs=10) as gat,
            tc.tile_pool(name="sel", bufs=8) as sel,
            tc.tile_pool(name="ep", bufs=3) as ep,
            tc.tile_pool(name="psum", bufs=2, space="PSUM") as psum,
        ):
            # ---- persistent tiles
            dslot_sb = meta.tile([128, C], F32)
            ewc_sb = meta.tile([128, C], F32)
            srcp1_sb = meta.tile([128, C], I32)
            srcp2_sb = meta.tile([128, C], I32)
            iota_sb = meta.tile([128, 128], BF)
            dis_sb = meta.tile([128, NBLK], F32)
            deg_sb = meta.tile([128, NBLK], F32)
            w1bf = [meta.tile([128, D], BF, tag=f"w1_{k}", name=f"w1bf{k}") for k in range(2)]
            w2bf = [meta.tile([128, D], BF, tag=f"w2_{k}", name=f"w2bf{k}") for k in range(2)]
            bc_tiles = {}
            for nm, src_t in (("b1", b1bc), ("g1", g1bc), ("e1", e1bc),
                              ("b2", b2bc), ("g2", g2bc), ("e2", e2bc)):
                bt = meta.tile([128, D], F32, tag=f"bc_{nm}", name=f"bc_{nm}")
                nc.sync.dma_start(bt[:], src_t[:, :])
                bc_tiles[nm] = bt
            ident = meta.tile([128, 128], BF)
            from concourse.masks import make_identity
            make_identity(nc, ident[:])
            one_sb = meta.tile([128, 1], F32)
            eps_sb = meta.tile([128, 1], F32)
            nc.vector.memset(one_sb[:], 1.0)
            nc.vector.memset(eps_sb[:], LN_EPS)

            nc.sync.dma_start(dslot_sb[:], dslot[:, :])
            nc.sync.dma_start(ewc_sb[:], ewraw[:, :])
            nc.scalar.activation(ewc_sb[:], ewc_sb[:], AF.Abs)
            nc.sync.dma_start(srcp1_sb[:], srcp1[:, :])
            nc.sync.dma_start(srcp2_sb[:], srcp2[:, :])
            nc.sync.dma_start(iota_sb[:], iota_in[:, :])
            for k in range(2):
                wtmp = ep.tile([128, D], F32, tag="wtmp")
                nc.sync.dma_start(wtmp[:], W1_in[k * 128:(k + 1) * 128, :])
                nc.scalar.activation(w1bf[k][:], wtmp[:], AF.Copy)
                wtmp2 = ep.tile([128, D], F32, tag="wtmp")
                nc.sync.dma_start(wtmp2[:], W2_in[k * 128:(k + 1) * 128, :])
                nc.scalar.activation(w2bf[k][:], wtmp2[:], AF.Copy)

            # ---- phase 0: degrees -> dis
            SLAB = 8
            for g0 in range(0, NBLK, SLAB):
                et = ellp.tile([128, SLAB, KDEG], F32, tag="ell")
                nc.sync.dma_start(
                    et[:], ell[:, g0 * KDEG:(g0 + SLAB) * KDEG]
                )
                nc.vector.tensor_reduce(
                    deg_sb[:, g0:g0 + SLAB], et[:], AX.X, AO.add,
                    apply_absolute_value=True,
                )
            nc.scalar.activation(dis_sb[:], deg_sb[:], AF.Sqrt, bias=one_sb[:])
            nc.vector.reciprocal(dis_sb[:], dis_sb[:])

            # ---- phase A1: full t1 = dis * (x @ W1), bf16
            for g in range(NBLK):
                ps = psum.tile([128, D], F32, tag="a1ps")
                for k in range(2):
                    xs = a1p.tile([128, 128], F32, tag="xs")
                    nc.sync.dma_start(xs[:], xTb[g, k * 128:(k + 1) * 128, :])
                    xsb = a1p.tile([128, 128], BF, tag="xsb")
                    nc.scalar.activation(xsb[:], xs[:], AF.Copy)
                    nc.tensor.matmul(
                        ps[:], lhsT=xsb[:], rhs=w1bf[k][:],
                        start=(k == 0), stop=(k == 1),
                    )
                xwo = a1p.tile([128, D], BF, tag="xwo")
                nc.vector.tensor_scalar(
                    out=xwo[:], in0=ps[:], scalar1=dis_sb[:, g:g + 1],
                    scalar2=None, op0=AO.mult,
                )
                nc.sync.dma_start(t1[g * 128:(g + 1) * 128, :], xwo[:])

            # ---- B loops
            def agg_layer(table, srcp_sb, layer):
                bias_t = bc_tiles["b1" if layer == 1 else "b2"]
                gain_t = bc_tiles["g1" if layer == 1 else "g2"]
                beta_t = bc_tiles["e1" if layer == 1 else "e2"]
                selfsrc = t1 if layer == 1 else xws2
                res_src = x_own if layer == 1 else h1
                out_dst = h1 if layer == 1 else h_out

                for b in range(BPC):
                    ps = psum.tile([128, D], F32, tag="agg")
                    for j in range(K):
                        cc = b * K + j
                        gt = gat.tile([128, D], BF, tag="g")
                        nc.gpsimd.indirect_dma_start(
                            out=gt[:], out_offset=None, in_=table[:, :],
                            in_offset=IndirectOffsetOnAxis(
                                ap=srcp_sb[:, cc:cc + 1], axis=0
                            ),
                        )
                        st = sel.tile([128, 128], BF, tag="s")
                        nc.vector.tensor_scalar(
                            out=st[:], in0=iota_sb[:],
                            scalar1=dslot_sb[:, cc:cc + 1],
                            scalar2=ewc_sb[:, cc:cc + 1],
                            op0=AO.is_equal, op1=AO.mult,
                        )
                        nc.tensor.matmul(
                            ps[:], lhsT=st[:], rhs=gt[:],
                            start=(j == 0), stop=(j == K - 1),
                        )
                    # epilogue
                    r0, r1 = b * 128, (b + 1) * 128
                    xwsv = ep.tile([128, D], BF, tag="xwsv")
                    nc.sync.dma_start(xwsv[:], selfsrc[r0:r1, :])
                    tf = ep.tile([128, D], F32, tag="tf")
                    nc.vector.tensor_tensor(
                        out=tf[:], in0=ps[:], in1=xwsv[:], op=AO.add
                    )
                    z2 = ep.tile([128, D], F32, tag="z2")
                    nc.vector.scalar_tensor_tensor(
                        out=z2[:], in0=tf[:], scalar=dis_sb[:, b:b + 1],
                        in1=bias_t[:], op0=AO.mult, op1=AO.add,
                    )
                    st6 = ep.tile([128, 6], F32, tag="st6")
                    nc.vector.bn_stats(st6[:], z2[:])
                    mv = ep.tile([128, 2], F32, tag="mv")
                    nc.vector.bn_aggr(mv[:], st6[:])
                    sd = ep.tile([128, 1], F32, tag="sd")
                    nc.scalar.activation(sd[:], mv[:, 1:2], AF.Sqrt, bias=eps_sb[:])
                    rstd = ep.tile([128, 1], F32, tag="rstd")
                    nc.vector.reciprocal(rstd[:], sd[:])
                    y = ep.tile([128, D], F32, tag="y")
                    nc.vector.tensor_scalar(
                        out=y[:], in0=z2[:], scalar1=mv[:, 0:1], scalar2=rstd[:],
                        op0=AO.subtract, op1=AO.mult,
                    )
                    y2 = ep.tile([128, D], F32, tag="y2")
                    nc.vector.tensor_tensor(out=y2[:], in0=y[:], in1=gain_t[:], op=AO.mult)
                    y3 = ep.tile([128, D], F32, tag="y3")
                    nc.vector.tensor_tensor(out=y3[:], in0=y2[:], in1=beta_t[:], op=AO.add)
                    r = ep.tile([128, D], F32, tag="r")
                    nc.scalar.activation(r[:], y3[:], AF.Relu)
                    hres = ep.tile([128, D], F32, tag="hres")
                    nc.sync.dma_start(hres[:], res_src[r0:r1, :])
                    ho = ep.tile([128, D], F32, tag="ho")
                    nc.vector.tensor_tensor(out=ho[:], in0=r[:], in1=hres[:], op=AO.add)
                    nc.sync.dma_start(out_dst[r0:r1, :], ho[:])

                    if layer == 1:
                        hb = ep.tile([128, D], BF, tag="hb")
                        nc.scalar.activation(hb[:], ho[:], AF.Copy)
                        ps2 = psum.tile([128, D], F32, tag="xw2")
                        for k in range(2):
                            pst = psum.tile([128, 128], BF, tag="tr")
                            nc.tensor.transpose(
                                pst[:], hb[:, k * 128:(k + 1) * 128], ident[:]
                            )
                            hT = ep.tile([128, 128], BF, tag=f"hT{k}")
                            nc.vector.tensor_copy(hT[:], pst[:])
                            nc.tensor.matmul(
                                ps2[:], lhsT=hT[:], rhs=w2bf[k][:],
                                start=(k == 0), stop=(k == 1),
                            )
                        x2o = ep.tile([128, D], BF, tag="x2o")
                        nc.vector.tensor_scalar(
                            out=x2o[:], in0=ps2[:], scalar1=dis_sb[:, b:b + 1],
                            scalar2=None, op0=AO.mult,
                        )
                        nc.sync.dma_start(xws2[r0:r1, :], x2o[:])
                        nc.sync.dma_start(xws2d[r0:r1, :], x2o[:])

            agg_layer(t1, srcp1_sb, 1)
            nc.sync.dma_start(dis_dump[:, :], dis_sb[:])

            # AllGather layer-2 table
            nc.gpsimd.collective_compute(
                "AllGather", mybir.AluOpType.bypass,
                replica_groups=[list(range(NC))],
                ins=[xws2[:, :]], outs=[t2[:, :]],
            )

            agg_layer(t2, srcp2_sb, 2)

    nc.compile()
    return nc


def _install_ntff_hook():
    import types
    if "antenv.axon_hooks" not in sys.modules:
        try:
            import antenv
        except ImportError:
            return
        mod = types.ModuleType("antenv.axon_hooks")
        mod._hook = None
        def set_axon_ntff_profile_hook(h):
            mod._hook = h
        def get_axon_ntff_profile_hook():
            return mod._hook
        mod.set_axon_ntff_profile_hook = set_axon_ntff_profile_hook
        mod.get_axon_ntff_profile_hook = get_axon_ntff_profile_hook
        sys.modules["antenv.axon_hooks"] = mod
        antenv.axon_hooks = mod
    try:
        sys.path.insert(0, "/root/.axon_site")
        from trn_agent_boot.trn_boot import _ntff_profile_via_ctypes
        hook = _ntff_profile_via_ctypes("/opt/axon/libaxon_pjrt.so")
        if hook is not None:
            sys.modules["antenv.axon_hooks"].set_axon_ntff_profile_hook(hook)
        import concourse.bass_utils as bu
        bu.upload_artifacts = lambda tmpdir: ""
    except Exception:
        pass


def kernel(x, edge_index, edge_weight, W1, b1, ln_g1, ln_b1, W2, b2, ln_g2,
           ln_b2):
    global LAST_RESULT
    from concourse.bass_utils import run_bass_kernel_spmd

    if os.environ.get("BASS_TRACE"):
        _install_ntff_hook()

    if "pk" not in _cache:
        _cache["pk"] = _pack(edge_index)
    pk = _cache["pk"]
    if "nc" not in _cache:
        _cache["nc"] = _build_program()
    nc = _cache["nc"]

    host_ins = _host_inputs(pk, x, edge_weight)
    iota_np = np.tile(np.arange(128, dtype=np.float32), (128, 1)).astype(
        ml_dtypes.bfloat16
    )
    shared = dict(
        iota=iota_np,
        W1=np.asarray(W1, np.float32), W2=np.asarray(W2, np.float32),
        b1bc=np.broadcast_to(np.asarray(b1, np.float32), (128, D)).copy(),
        g1bc=np.broadcast_to(np.asarray(ln_g1, np.float32), (128, D)).copy(),
        e1bc=np.broadcast_to(np.asarray(ln_b1, np.float32), (128, D)).copy(),
        b2bc=np.broadcast_to(np.asarray(b2, np.float32), (128, D)).copy(),
        g2bc=np.broadcast_to(np.asarray(ln_g2, np.float32), (128, D)).copy(),
        e2bc=np.broadcast_to(np.asarray(ln_b2, np.float32), (128, D)).copy(),
    )
    in_maps = []
    for c in range(NC):
        m = dict(shared)
        hi = host_ins[c]
        m.update(
            xTb=hi["xTb"], x_own=hi["x_own"], dslot=hi["dslot"],
            ewraw=hi["ewraw"], srcp1=hi["srcp1"], srcp2=hi["srcp2"],
            ell=hi["ell"],
        )
        in_maps.append(m)

    res = run_bass_kernel_spmd(nc, in_maps, list(range(NC)))
    LAST_RESULT = res

    # reassemble: shard row b*128+s of core c -> node at (c, b, s)
    core_of, block_of, slot_of = pk["core_of"], pk["block_of"], pk["slot_of"]
    out = np.empty((N, D), np.float32)
    loc = block_of * 128 + slot_of
    for c in range(NC):
        mask = core_of == c
        out[mask] = res.results[c]["h_out"][loc[mask]]
    return out

